# revision 2
# baseline (speedup 1.0000x reference)
"""Trainium2 Bass kernel for nn_Attention_73031623901249.

Multi-head attention with per-head 512x512 projections, interleaved RoPE,
causal softmax, a transposed P^T @ V contraction, and an output projection.

Sharding: one head per NeuronCore (H == 8 == n_cores). Each core computes its
head's full attention; the host sums the 8 partial outputs.

v2 structural changes over the first working version:
  - W_o is folded into W_v on the host (associativity:
    (P^T (q W_v)) W_o == P^T (q (W_v W_o))), deleting the whole on-chip
    W_o stage (128 matmuls + 64 PSUM evacuation copies per batch pair).
  - Scores run as fp8e4m3 DoubleRow matmuls (2 d-tile pairs per
    instruction, 0.5 cyc/row): rope'd Q^T/K^T are written as fp8 into
    [128, 2, S] pair tiles. Q is pre-scaled by 2^10/sqrt(D) and K by 2^6
    so fp8 quantization error stays relative (max |Q| ~ 142, |K| ~ 174,
    both < 240); the 2^-16 descale rides the Exp activation's scale.
  - Softmax reciprocals batched 4-at-a-time ([128,4] per group) instead
    of 16 tiny [128,1] ops.

Layout choices (host-side prep):
  - q is fed transposed as qT [D, B*S] so projections need no on-chip
    transposes.
  - W_q / W_k columns are permuted even/odd -> [evens | odds], which turns
    interleaved RoPE into elementwise ops on partition-aligned halves.
    Scores are permutation-invariant.
  - cos/sin tables are fed as [D/2, S] (transposed), computed with numpy
    float32 to match the reference bit-for-bit.
  - The softmax denominator is folded into V as a per-row scale (the
    contraction index of P^T @ V is the softmax-row index).

Scores rows are computed at their exact causal width; the triangular mask
for the diagonal 128-block is added on the PE (ident^T @ mask). Scores are
small by construction (|s| < ~2), so exp runs without max-subtraction. The
P^T V stage accumulates each output chunk in descending-t order so the
first (widest) matmul covers the whole PSUM bank and every later write
lands on already-written columns.
"""

import sys

if "/opt/trn_rl_repo" not in sys.path:
    sys.path.insert(0, "/opt/trn_rl_repo")

import math

import numpy as np

import concourse.bacc as bacc
import concourse.tile as tile
from concourse import mybir

F32 = mybir.dt.float32
F32R = mybir.dt.float32r
F8 = mybir.dt.float8e4
AX = mybir.AxisListType
AF = mybir.ActivationFunctionType
DR = mybir.MatmulPerfMode.DoubleRow

B, S, D, H = 2, 2048, 512, 8
NCORES = 8
NEG = -1.0e30  # additive causal mask value

QSHIFT = 10  # Q side pre-scale 2^10 (on top of 1/sqrt(D))
KSHIFT = 6   # K side pre-scale 2^6
EXP_SCALE = 2.0 ** (-(QSHIFT + KSHIFT))

_BUILT = None


def build_kernel(reps=1):
    nc = bacc.Bacc(trn_type="TRN2", target_bir_lowering=False, debug=False)

    qT_d = nc.dram_tensor("qT", [D, B * S], F32, kind="ExternalInput").ap()
    wq_d = nc.dram_tensor("wq", [D, D], F32, kind="ExternalInput").ap()
    wk_d = nc.dram_tensor("wk", [D, D], F32, kind="ExternalInput").ap()
    wv_d = nc.dram_tensor("wv", [D, D], F32, kind="ExternalInput").ap()
    cos_d = nc.dram_tensor("cos2", [D // 2, S], F32, kind="ExternalInput").ap()
    sin_d = nc.dram_tensor("sin2", [D // 2, S], F32, kind="ExternalInput").ap()
    mask_d = nc.dram_tensor("mask1", [128, 128], mybir.dt.bfloat16,
                            kind="ExternalInput").ap()
    ident_d = nc.dram_tensor("ident", [128, 128], mybir.dt.bfloat16,
                             kind="ExternalInput").ap()
    outT_d = nc.dram_tensor("outT", [B, D, S], F32, kind="ExternalOutput").ap()

    NT = S // 128  # 16 q/key tiles per batch

    with tile.TileContext(nc) as tc:
        with tc.tile_pool(name="const", bufs=1) as constp:
            wq_sb, wk_sb, wv_sb = [], [], []
            for nm, lst in (("wq", wq_sb), ("wk", wk_sb), ("wv", wv_sb)):
                for zt in range(4):
                    lst.append(constp.tile([128, D], F32R, name=f"{nm}{zt}"))
            mask_sb = constp.tile([128, 128], mybir.dt.bfloat16, name="mask_sb")
            ident_sb = constp.tile([128, 128], mybir.dt.bfloat16,
                                   name="ident_sb")
            # only wq's loads go first — the rest are emitted inside batch 0
            # (see deferred_loads) so they don't crowd the DMA queues ahead of
            # the first projection's qT slices
            for zt in range(4):
                nc.sync.dma_start(
                    out=wq_sb[zt],
                    in_=wq_d[128 * zt : 128 * (zt + 1), :].bitcast(F32R),
                )

            def deferred_loads(stage):
                if stage == 0:
                    for zt in range(4):
                        nc.sync.dma_start(
                            out=wk_sb[zt],
                            in_=wk_d[128 * zt : 128 * (zt + 1), :].bitcast(F32R),
                        )
                elif stage == 1:
                    for zt in range(4):
                        nc.sync.dma_start(
                            out=wv_sb[zt],
                            in_=wv_d[128 * zt : 128 * (zt + 1), :].bitcast(F32R),
                        )
                    nc.sync.dma_start(out=mask_sb, in_=mask_d)
                    nc.sync.dma_start(out=ident_sb, in_=ident_d)

            for _rep in range(reps):
                for b in range(B):
                    _build_batch(
                        nc, tc, b, qT_d, wq_sb, wk_sb, wv_sb, cos_d,
                        sin_d, mask_sb, ident_sb, outT_d, NT,
                        deferred_loads if (_rep == 0 and b == 0) else None,
                    )
    nc.compile()
    return nc


def _build_batch(nc, tc, b, qT_d, wq_sb, wk_sb, wv_sb, cos_d, sin_d,
                 mask_sb, ident_sb, outT_d, NT, deferred_loads=None):
    with (
        tc.tile_pool(name=f"qk{b}", bufs=1) as qkpool,
        tc.tile_pool(name=f"v{b}", bufs=1) as vpool,
        tc.tile_pool(name=f"misc{b}", bufs=1) as mpool,
        tc.tile_pool(name=f"p0{b}", bufs=1) as ppool0,
    ):
        # rope'd Q^T/K^T as fp8 pair tiles: QTp[g][:, i, :] is d-tile pair
        # member i of group g (g=0: cos-combined halves, g=1: sin-combined)
        QTp = [qkpool.tile([128, 2, S], F8, name=f"b{b}QTp{g}", tag=f"QTp{g}")
               for g in range(2)]
        KTp = [qkpool.tile([128, 2, S], F8, name=f"b{b}KTp{g}", tag=f"KTp{g}")
               for g in range(2)]
        V = [vpool.tile([128, D], F32R, name=f"b{b}V{t}", tag=f"V{t}")
             for t in range(NT)]
        rsum = mpool.tile([128, NT], F32, name=f"b{b}rsum")
        rinv = mpool.tile([128, NT], F32, name=f"b{b}rinv")

        P = []

        def emit_scores(t, ps, pool_p):
            Kt = 128 * (t + 1)
            nch = t // 4 + 1
            for g in range(2):
                for c in range(nch):
                    sl = slice(512 * c, min(512 * (c + 1), Kt))
                    nc.tensor.matmul(
                        ps[:, sl],
                        QTp[g][:, :, 128 * t : 128 * (t + 1)],
                        KTp[g][:, :, sl],
                        start=(g == 0),
                        stop=(g == 1 and c < nch - 1),
                        perf_mode=DR,
                    )
            # additive triangular mask on the diagonal block via the PE
            nc.tensor.matmul(
                ps[:, Kt - 128 : Kt], ident_sb, mask_sb,
                start=False, stop=True,
            )
            p_t = pool_p.tile([128, Kt], F32R, name=f"b{b}p{t}", tag=f"p{t}")
            nc.scalar.activation(
                p_t, ps[:, :Kt], AF.Exp, scale=EXP_SCALE,
                accum_out=rsum[:, t : t + 1],
            )
            P.append(p_t)

        def finish_group(j):
            # rows 4j..4j+3 have their rsum; fold 1/rsum into V rows
            nc.vector.reciprocal(rinv[:, 4 * j : 4 * j + 4],
                                 rsum[:, 4 * j : 4 * j + 4])
            for k in range(4):
                t = 4 * j + k
                nc.vector.tensor_scalar_mul(V[t], V[t], rinv[:, t : t + 1])

        # ---------------- phase 1: projections + rope ----------------
        with (
            tc.tile_pool(name=f"st{b}", bufs=2) as spool,
            tc.tile_pool(name=f"t{b}", bufs=2) as tpool,
            tc.tile_pool(name=f"psA{b}", bufs=2, space="PSUM") as psA,
        ):
            for j in range(4):  # 512-wide s-chunks of this batch
                c0 = b * S + 512 * j
                qs = []
                for zt in range(4):
                    t_ = spool.tile([128, 512], F32R, name=f"b{b}qs{zt}_{j}",
                                    tag=f"qs{zt}")
                    nc.sync.dma_start(
                        out=t_,
                        in_=qT_d[128 * zt : 128 * (zt + 1),
                                 c0 : c0 + 512].bitcast(F32R),
                    )
                    qs.append(t_)
                trig = {}
                for nm, dram in (("c", cos_d), ("s", sin_d)):
                    for i in range(2):
                        t_ = spool.tile([128, 512], F32, name=f"b{b}{nm}{i}_{j}",
                                        tag=f"tr{nm}{i}")
                        nc.sync.dma_start(
                            out=t_,
                            in_=dram[128 * i : 128 * (i + 1),
                                     512 * j : 512 * (j + 1)],
                        )
                        trig[nm, i] = t_

                # Q and K projections with rope applied on the way to SBUF
                for nm, wsb, dst in (("q", wq_sb, QTp), ("k", wk_sb, KTp)):
                    if deferred_loads is not None and nm == "k" and j == 0:
                        deferred_loads(0)
                    for i in range(2):  # pair-half index
                        pe = psA.tile([128, 512], F32, name=f"b{b}{nm}pe{i}_{j}",
                                      tag="pe", space="PSUM")
                        po = psA.tile([128, 512], F32, name=f"b{b}{nm}po{i}_{j}",
                                      tag="po", space="PSUM")
                        for zt in range(4):
                            nc.tensor.matmul(
                                pe, wsb[zt][:, 128 * i : 128 * (i + 1)], qs[zt],
                                start=(zt == 0), stop=(zt == 3),
                            )
                        for zt in range(4):
                            nc.tensor.matmul(
                                po, wsb[zt][:, 128 * (i + 2) : 128 * (i + 3)],
                                qs[zt], start=(zt == 0), stop=(zt == 3),
                            )
                        sl = slice(512 * j, 512 * (j + 1))
                        t1 = tpool.tile([128, 512], F32, name=f"t1_{b}{nm}{i}{j}",
                                        tag="t1")
                        t2 = tpool.tile([128, 512], F32, name=f"t2_{b}{nm}{i}{j}",
                                        tag="t2")
                        nc.vector.tensor_mul(t1, pe, trig["c", i])
                        nc.vector.tensor_mul(t2, po, trig["s", i])
                        nc.gpsimd.tensor_sub(dst[0][:, i, sl], t1, t2)
                        t3 = tpool.tile([128, 512], F32, name=f"t3_{b}{nm}{i}{j}",
                                        tag="t3")
                        t4 = tpool.tile([128, 512], F32, name=f"t4_{b}{nm}{i}{j}",
                                        tag="t4")
                        nc.vector.tensor_mul(t3, pe, trig["s", i])
                        nc.vector.tensor_mul(t4, po, trig["c", i])
                        nc.gpsimd.tensor_add(dst[1][:, i, sl], t3, t4)

                if deferred_loads is not None and j == 0:
                    deferred_loads(1)
                    deferred_loads = None
                # V projection (natural [s, d] layout; qT slices as stationary)
                for st in range(4):
                    pv = psA.tile([128, 512], F32, name=f"b{b}pv{j}_{st}",
                                  tag="pv", space="PSUM")
                    for zt in range(4):
                        nc.tensor.matmul(
                            pv, qs[zt][:, 128 * st : 128 * (st + 1)], wv_sb[zt],
                            start=(zt == 0), stop=(zt == 3),
                        )
                    nc.scalar.copy(V[4 * j + st], pv)

            # rows t=0..3 are <=512 wide: run them on the 2 PSUM banks the
            # projection pool never owned, overlapping the phase-1 drain
            with tc.tile_pool(name=f"psS0{b}", bufs=2, space="PSUM") as psS0:
                for t in range(4):
                    ps = psS0.tile([128, 512], F32, name=f"b{b}ps{t}",
                                   tag="s0", space="PSUM")
                    emit_scores(t, ps, ppool0)
                finish_group(0)

        # ---------------- phase 2: scores + softmax ----------------
        with tc.tile_pool(name=f"p{b}", bufs=1) as ppool:
            with tc.tile_pool(name=f"psS{b}", bufs=2, space="PSUM") as psS:
                for t in range(4, NT):
                    ps = psS.tile([128, S], F32, name=f"b{b}ps{t}", tag="s",
                                  space="PSUM")
                    emit_scores(t, ps, ppool)
                    if t % 4 == 3:
                        finish_group(t // 4)

            # ------------- phase 3: out^T = V^T P (W_o pre-folded) -------------
            with (
                tc.tile_pool(name=f"o{b}", bufs=2) as opool,
                tc.tile_pool(name=f"psPV{b}", bufs=1, space="PSUM") as psPV,
            ):
                for j in range(4):
                    po = [psPV.tile([128, 512], F32, name=f"b{b}po{j}_{dt_}",
                                    tag=f"o{dt_}", space="PSUM")
                          for dt_ in range(4)]
                    # the first matmul must cover the whole bank (uniform
                    # fresh-write); pick the EARLIEST full-width t (4j+3) so
                    # this chunk's accumulation can begin before the last
                    # softmax rows finish, then take the remaining t in any
                    # order (all later writes land on written columns)
                    order = [4 * j + 3] + list(range(4 * j + 4, NT)) + [
                        4 * j + 2, 4 * j + 1, 4 * j]
                    for t in order:
                        n = min(512, 128 * (t + 1) - 512 * j)
                        for dt_ in range(4):
                            nc.tensor.matmul(
                                po[dt_][:, :n],
                                V[t][:, 128 * dt_ : 128 * (dt_ + 1)],
                                P[t][:, 512 * j : 512 * j + n],
                                start=(t == order[0]), stop=(t == order[-1]),
                            )
                    for dt_ in range(4):
                        of = opool.tile([128, 512], F32, name=f"b{b}of{j}_{dt_}",
                                        tag=f"of{dt_}")
                        nc.vector.tensor_copy(of, po[dt_])
                        nc.sync.dma_start(
                            out=outT_d[b, 128 * dt_ : 128 * (dt_ + 1),
                                       512 * j : 512 * (j + 1)],
                            in_=of,
                        )


def _host_inputs(q, W_q, W_k, W_v, W_o):
    """Build the 8 per-core input maps."""
    scale = 1.0 / math.sqrt(D)
    perm = np.concatenate([np.arange(0, D, 2), np.arange(1, D, 2)])

    qT = np.ascontiguousarray(q.reshape(B * S, D).T)  # [D, B*S]

    # trig tables, float32 pipeline mirroring the reference's jnp math
    inv_freq = (1.0 / (10000.0 ** (np.arange(0, D, 2, dtype=np.float32) /
                                   np.float32(D)))).astype(np.float32)
    ang = (np.arange(S, dtype=np.float32)[:, None] * inv_freq[None, :])
    cos2 = np.ascontiguousarray(np.cos(ang, dtype=np.float32).T)
    sin2 = np.ascontiguousarray(np.sin(ang, dtype=np.float32).T)

    # additive triangular mask for the diagonal 128x128 block
    import ml_dtypes
    r = np.arange(128)[:, None]
    c = np.arange(128)[None, :]
    mask1 = np.where(c <= r, 0.0, NEG).astype(ml_dtypes.bfloat16)
    ident = np.eye(128, dtype=ml_dtypes.bfloat16)

    in_maps = []
    for h in range(NCORES):
        # fold W_o into W_v: (P^T (q W_v)) W_o == P^T (q (W_v W_o))
        wv_folded = (
            W_v[h].astype(np.float64) @ W_o[D * h : D * (h + 1), :].astype(np.float64)
        ).astype(np.float32)
        in_maps.append({
            "qT": qT,
            "wq": np.ascontiguousarray(
                (W_q[h] * (scale * 2.0 ** QSHIFT))[:, perm]),
            "wk": np.ascontiguousarray((W_k[h] * 2.0 ** KSHIFT)[:, perm]),
            "wv": np.ascontiguousarray(wv_folded),
            "cos2": cos2,
            "sin2": sin2,
            "mask1": mask1,
            "ident": ident,
        })
    return in_maps


def kernel(q, W_q, W_k, W_v, W_o):
    from concourse.bass_utils import run_bass_kernel_spmd

    global _BUILT
    q = np.asarray(q, dtype=np.float32)
    W_q = np.asarray(W_q, dtype=np.float32)
    W_k = np.asarray(W_k, dtype=np.float32)
    W_v = np.asarray(W_v, dtype=np.float32)
    W_o = np.asarray(W_o, dtype=np.float32)

    if _BUILT is None:
        _BUILT = build_kernel()
    nc = _BUILT

    in_maps = _host_inputs(q, W_q, W_k, W_v, W_o)
    res = run_bass_kernel_spmd(nc, in_maps, list(range(NCORES)))

    acc = np.zeros((B, S, D), dtype=np.float64)
    for h in range(NCORES):
        acc += res.results[h]["outT"].transpose(0, 2, 1)
    return acc.astype(np.float32)


# revision 29
# speedup vs baseline: 1.0649x; 1.0649x over previous
"""Trainium2 Bass kernel for nn_Attention_73031623901249.

Multi-head attention with per-head 512x512 projections, interleaved RoPE,
causal softmax, a transposed P^T @ V contraction, and an output projection.

Sharding: one head per NeuronCore (H == 8 == n_cores). Each core computes its
head's full attention; the host sums the 8 partial outputs.

Structure (v6):
  - W_o folded into W_v on the host (associativity:
    (P^T (q W_v)) W_o == P^T (q (W_v W_o))) — no on-chip W_o stage.
  - q^T, W_q, W_k, W_v(folded), cos/sin are bf16; q^T and trig are loaded
    ONCE and stay resident in SBUF (no per-batch input DMAs).
  - Scores are fp8e4m3 DoubleRow matmuls (two d-tile pairs per
    instruction): rope'd Q^T/K^T are written as fp8 into [128, 2, S] pair
    tiles. Q is pre-scaled by 2^10/sqrt(D) and K by 2^6 (keeps fp8 values
    in the normal range, max ~174 < 240); the 2^-16 descale rides the Exp
    activation's scale input.
  - RoPE path: pe/po PSUM tiles are evacuated to bf16 by ACT copies, the
    four per-half products run on DVE in 2x bf16 mode, the two combines on
    GpSimd write fp8 directly.
  - P (softmax numerator) and V are bf16; the softmax denominator is folded
    into V as a per-row scale (the contraction index of P^T @ V is the
    softmax-row index). Reciprocals batched 4 rows at a time on DVE.
  - P^T V runs with P slices as the STATIONARY operand (V moving): one
    full-width [128,512] matmul per (s-tile u, z-tile t'), u <= t'. This
    4.7x's stationary reuse vs the V-stationary form, makes every matmul
    full width, and yields the output in natural [s, d] layout. The first
    4 s-tiles accumulate DURING the softmax phase (psS holds 4 PSUM banks,
    the 4 u-tiles the other 4); s-tiles 4..15 run after.

Scores rows are computed at their exact causal width; the triangular mask
for the diagonal 128-block is added on the PE (ident^T @ mask). Scores are
small by construction (|s| < ~2 after descale), so exp runs without
max-subtraction.
"""

import sys

if "/opt/trn_rl_repo" not in sys.path:
    sys.path.insert(0, "/opt/trn_rl_repo")

import math

import numpy as np

import concourse.bacc as bacc
import concourse.tile as tile
from concourse import mybir

F32 = mybir.dt.float32
BF16 = mybir.dt.bfloat16
F8 = mybir.dt.float8e4
AX = mybir.AxisListType
AF = mybir.ActivationFunctionType
DR = mybir.MatmulPerfMode.DoubleRow

B, S, D, H = 2, 2048, 512, 8
NCORES = 8
NEG = -1.0e30  # additive causal mask value

QSHIFT = 10  # Q side pre-scale 2^10 (on top of 1/sqrt(D))
KSHIFT = 6   # K side pre-scale 2^6
EXP_SCALE = 2.0 ** (-(QSHIFT + KSHIFT))

_BUILT = None


def build_kernel(reps=1):
    nc = bacc.Bacc(trn_type="TRN2", target_bir_lowering=False, debug=False)

    qT_d = nc.dram_tensor("qT", [D, B * S], BF16, kind="ExternalInput").ap()
    qT8_d = nc.dram_tensor("qT8", [D, B * S], F8, kind="ExternalInput").ap()
    wq_d = nc.dram_tensor("wq", [D, D], F8, kind="ExternalInput").ap()
    wk_d = nc.dram_tensor("wk", [D, D], F8, kind="ExternalInput").ap()
    wv_d = nc.dram_tensor("wv", [D, D], BF16, kind="ExternalInput").ap()
    cos_d = nc.dram_tensor("cos2", [D // 2, S], BF16, kind="ExternalInput").ap()
    sin_d = nc.dram_tensor("sin2", [D // 2, S], BF16, kind="ExternalInput").ap()
    mask_d = nc.dram_tensor("mask1", [128, 128], BF16,
                            kind="ExternalInput").ap()
    ident_d = nc.dram_tensor("ident", [128, 128], BF16,
                             kind="ExternalInput").ap()
    out_d = nc.dram_tensor("out", [B, S, D], F32, kind="ExternalOutput").ap()

    NT = S // 128  # 16 q/key tiles per batch

    with tile.TileContext(nc) as tc:
        with tc.tile_pool(name="const", bufs=1) as constp:
            # resident q^T: fp8 z-pair tiles (Q/K proj) + bf16 (V proj)
            q8p = [constp.tile([128, 2, B * S], F8, name=f"q8p{g}")
                   for g in range(2)]
            qs_all = [constp.tile([128, B * S], BF16, name=f"qs{zt}")
                      for zt in range(4)]
            # chunk-0 slices load first so the first projection starts ASAP
            for g in range(2):
                for m in range(2):
                    r0 = 256 * g + 128 * m
                    nc.sync.dma_start(
                        out=q8p[g][:, m, 0:512],
                        in_=qT8_d[r0 : r0 + 128, 0:512],
                    )
            for zt in range(4):
                nc.sync.dma_start(
                    out=qs_all[zt][:, 0:512],
                    in_=qT_d[128 * zt : 128 * (zt + 1), 0:512],
                )
            wq8p = [constp.tile([128, 2, D], F8, name=f"wq8p{g}")
                    for g in range(2)]
            wk8p = [constp.tile([128, 2, D], F8, name=f"wk8p{g}")
                    for g in range(2)]
            wv_sb = [constp.tile([128, D], BF16, name=f"wv{zt}")
                     for zt in range(4)]
            trig = {}
            for nm in ("c", "s"):
                for i in range(2):
                    trig[nm, i] = constp.tile([128, S], BF16,
                                              name=f"trig{nm}{i}")
            mask_sb = constp.tile([128, 128], BF16, name="mask_sb")
            ident_sb = constp.tile([128, 128], BF16, name="ident_sb")
            # weights on the ACT queue and trig on the GpSimd (SWDGE) queue
            # so the SP queue carries only q — three DMA streams run
            # concurrently
            for g in range(2):
                for m in range(2):
                    r0 = 256 * g + 128 * m
                    nc.scalar.dma_start(
                        out=wq8p[g][:, m, :], in_=wq_d[r0 : r0 + 128, :],
                    )
            for nm, dram in (("c", cos_d), ("s", sin_d)):
                for i in range(2):
                    nc.gpsimd.dma_start(
                        out=trig[nm, i],
                        in_=dram[128 * i : 128 * (i + 1), :],
                    )
            for g in range(2):
                for m in range(2):
                    r0 = 256 * g + 128 * m
                    nc.sync.dma_start(
                        out=q8p[g][:, m, 512:S],
                        in_=qT8_d[r0 : r0 + 128, 512:S],
                    )
            for zt in range(4):
                nc.sync.dma_start(
                    out=qs_all[zt][:, 512:S],
                    in_=qT_d[128 * zt : 128 * (zt + 1), 512:S],
                )

            def deferred_loads(stage):
                if stage == 0:
                    for g in range(2):
                        for m in range(2):
                            r0 = 256 * g + 128 * m
                            nc.scalar.dma_start(
                                out=wk8p[g][:, m, :],
                                in_=wk_d[r0 : r0 + 128, :],
                            )
                elif stage == 1:
                    for zt in range(4):
                        nc.scalar.dma_start(
                            out=wv_sb[zt],
                            in_=wv_d[128 * zt : 128 * (zt + 1), :],
                        )
                    nc.gpsimd.dma_start(out=mask_sb, in_=mask_d)
                    nc.gpsimd.dma_start(out=ident_sb, in_=ident_d)
                    # batch-1 half of q
                    for g in range(2):
                        for m in range(2):
                            r0 = 256 * g + 128 * m
                            nc.sync.dma_start(
                                out=q8p[g][:, m, S : 2 * S],
                                in_=qT8_d[r0 : r0 + 128, S : 2 * S],
                            )
                    for zt in range(4):
                        nc.sync.dma_start(
                            out=qs_all[zt][:, S : 2 * S],
                            in_=qT_d[128 * zt : 128 * (zt + 1), S : 2 * S],
                        )

            for _rep in range(reps):
                for b in range(B):
                    _build_batch(
                        nc, tc, b, q8p, qs_all, wq8p, wk8p, wv_sb, trig,
                        mask_sb, ident_sb, out_d, NT,
                        deferred_loads if (_rep == 0 and b == 0) else None,
                    )
    nc.compile()
    return nc


def _build_batch(nc, tc, b, q8p, qs_all, wq8p, wk8p, wv_sb, trig,
                 mask_sb, ident_sb, out_d, NT, deferred_loads=None):
    with (
        tc.tile_pool(name=f"qk{b}", bufs=1) as qkpool,
        tc.tile_pool(name=f"v{b}", bufs=1) as vpool,
        tc.tile_pool(name=f"misc{b}", bufs=1) as mpool,
        tc.tile_pool(name=f"p0{b}", bufs=1) as ppool0,
    ):
        # rope'd Q^T/K^T as fp8 pair tiles: QTp[g][:, i, :] is d-tile pair
        # member i of group g (g=0: cos-combined halves, g=1: sin-combined)
        QTp = [qkpool.tile([128, 2, S], F8, name=f"b{b}QTp{g}", tag=f"QTp{g}")
               for g in range(2)]
        KTp = [qkpool.tile([128, 2, S], F8, name=f"b{b}KTp{g}", tag=f"KTp{g}")
               for g in range(2)]
        V = [vpool.tile([128, D], BF16, name=f"b{b}V{t}", tag=f"V{t}")
             for t in range(NT)]
        rsumA = mpool.tile([128, NT], F32, name=f"b{b}rsumA")
        rsumB = mpool.tile([128, NT], F32, name=f"b{b}rsumB")
        rsum = mpool.tile([128, NT], F32, name=f"b{b}rsum")
        rinv = mpool.tile([128, NT], F32, name=f"b{b}rinv")
        # rows whose scores fit one PSUM tile never write rsumB — zero it
        nc.vector.memset(rsumB, 0.0)

        P = []

        def emit_scores(t, pool_ps, pool_p, width=1024):
            """Score row t in half-row PSUM tiles (<=2 banks each)."""
            Kt = 128 * (t + 1)
            nch = (Kt + 511) // 512
            nca = min(nch, width // 512)  # chunks in tile A
            psa = pool_ps.tile([128, width], F32, name=f"b{b}psA{t}",
                               tag="s", space="PSUM")
            psb = None
            if nch > nca:
                psb = pool_ps.tile([128, width], F32, name=f"b{b}psB{t}",
                                   tag="s", space="PSUM")

            for g in range(2):
                for c in range(nch):
                    lo, hi = 512 * c, min(512 * (c + 1), Kt)
                    tl, loc0 = (psa, lo) if c < nca else (psb, lo - width)
                    nc.tensor.matmul(
                        tl[:, loc0 : loc0 + hi - lo],
                        QTp[g][:, :, 128 * t : 128 * (t + 1)],
                        KTp[g][:, :, lo:hi],
                        start=(g == 0),
                        stop=(g == 1 and c < nch - 1),
                        perf_mode=DR,
                    )
            # additive triangular mask on the diagonal block via the PE
            mtl, mloc = (psa, Kt - 128) if nch <= nca else (psb, Kt - 128 - width)
            nc.tensor.matmul(
                mtl[:, mloc : mloc + 128], ident_sb, mask_sb,
                start=False, stop=True,
            )
            p_t = pool_p.tile([128, Kt], BF16, name=f"b{b}p{t}", tag=f"p{t}")
            nA = min(Kt, width)
            nc.scalar.activation(
                p_t[:, :nA], psa[:, :nA], AF.Exp, scale=EXP_SCALE,
                accum_out=rsumA[:, t : t + 1],
            )
            if psb is not None:
                nc.scalar.activation(
                    p_t[:, width:Kt], psb[:, : Kt - width], AF.Exp,
                    scale=EXP_SCALE, accum_out=rsumB[:, t : t + 1],
                )
            P.append(p_t)

        def finish_group(j):
            # rows 4j..4j+3 have their partial sums; combine + invert (DVE)
            g4 = slice(4 * j, 4 * j + 4)
            nc.vector.tensor_add(rsum[:, g4], rsumA[:, g4], rsumB[:, g4])
            nc.vector.reciprocal(rinv[:, g4], rsum[:, g4])

        def scale_group(j):
            # fold 1/rsum into V rows on DVE — emitted only at points where
            # the DVE queue has no pending rope work (the phase boundary /
            # softmax window), so the strict-FIFO queue never stalls rope
            for k in range(4):
                t = 4 * j + k
                nc.vector.tensor_scalar_mul(V[t], V[t], rinv[:, t : t + 1])

        # ---------------- phase 1: projections + rope ----------------
        # score rows 0..7 are emitted INSIDE phase 1 (rows t need only the
        # first t//4+1 K-chunks), soaking up ACT slack so the later
        # softmax-bound window only covers rows 8..15.
        with (
            tc.tile_pool(name=f"t{b}", bufs=2) as tpool,
            tc.tile_pool(name=f"psA{b}", bufs=2, space="PSUM") as psA,
            tc.tile_pool(name=f"psV{b}", bufs=1, space="PSUM") as psV,
            tc.tile_pool(name=f"psS0{b}", bufs=1, space="PSUM") as psS0,
        ):
            def rope(nm, dst, i, j, pe, po):
                sl = slice(512 * j, 512 * (j + 1))
                peb = tpool.tile([128, 512], BF16,
                                 name=f"peb_{b}{nm}{i}{j}", tag="peb")
                pob = tpool.tile([128, 512], BF16,
                                 name=f"pob_{b}{nm}{i}{j}", tag="pob")
                nc.vector.tensor_copy(peb, pe)
                nc.vector.tensor_copy(pob, po)
                t1 = tpool.tile([128, 512], BF16,
                                name=f"t1_{b}{nm}{i}{j}", tag="t1")
                t2 = tpool.tile([128, 512], BF16,
                                name=f"t2_{b}{nm}{i}{j}", tag="t2")
                nc.vector.tensor_mul(t1, peb, trig["c", i][:, sl])
                nc.vector.tensor_mul(t2, pob, trig["s", i][:, sl])
                nc.gpsimd.tensor_sub(dst[0][:, i, sl], t1, t2)
                t3 = tpool.tile([128, 512], BF16,
                                name=f"t3_{b}{nm}{i}{j}", tag="t3")
                t4 = tpool.tile([128, 512], BF16,
                                name=f"t4_{b}{nm}{i}{j}", tag="t4")
                nc.vector.tensor_mul(t3, peb, trig["s", i][:, sl])
                nc.vector.tensor_mul(t4, pob, trig["c", i][:, sl])
                nc.gpsimd.tensor_add(dst[1][:, i, sl], t3, t4)

            for jp in range(2):  # chunk pairs: (0,1) then (2,3)
                js = (2 * jp, 2 * jp + 1)
                qs = {jj: [qs_all[zt][:, b * S + 512 * jj : b * S + 512 * jj + 512]
                           for zt in range(4)] for jj in js}

                # Q and K projections: fp8 DoubleRow over z-pairs (their
                # only consumer is the fp8 scores path). Both chunks of the
                # pair accumulate under one stationary load per (g, d-col).
                for nm, w8p, dst in (("q", wq8p, QTp), ("k", wk8p, KTp)):
                    if deferred_loads is not None and nm == "k" and jp == 0:
                        deferred_loads(0)
                    for i in range(2):  # pair-half index
                        pe = {jj: psA.tile([128, 512], F32,
                                           name=f"b{b}{nm}pe{i}_{jj}",
                                           tag="pe", space="PSUM")
                              for jj in js}
                        po = {jj: psA.tile([128, 512], F32,
                                           name=f"b{b}{nm}po{i}_{jj}",
                                           tag="po", space="PSUM")
                              for jj in js}
                        for g in range(2):
                            for jj in js:
                                c0 = b * S + 512 * jj
                                nc.tensor.matmul(
                                    pe[jj],
                                    w8p[g][:, :, 128 * i : 128 * (i + 1)],
                                    q8p[g][:, :, c0 : c0 + 512],
                                    start=(g == 0), stop=(g == 1),
                                    perf_mode=DR,
                                )
                        for g in range(2):
                            for jj in js:
                                c0 = b * S + 512 * jj
                                nc.tensor.matmul(
                                    po[jj],
                                    w8p[g][:, :, 128 * (i + 2) : 128 * (i + 3)],
                                    q8p[g][:, :, c0 : c0 + 512],
                                    start=(g == 0), stop=(g == 1),
                                    perf_mode=DR,
                                )
                        for jj in js:
                            rope(nm, dst, i, jj, pe[jj], po[jj])

                if deferred_loads is not None and jp == 0:
                    deferred_loads(1)
                    deferred_loads = None
                # V projection (natural [s, d] layout; qT slices as
                # stationary), interleaved with the early score rows in
                # pair 0: both K-chunks are rope'd by now, so rows 0..7
                # run here and their exps soak ACT slack during pair 1
                for jj in js:
                    for st in range(4):
                        pv = psV.tile([128, 512], F32, name=f"b{b}pv{jj}_{st}",
                                      tag="pv", space="PSUM")
                        for zt in range(4):
                            nc.tensor.matmul(
                                pv, qs[jj][zt][:, 128 * st : 128 * (st + 1)],
                                wv_sb[zt],
                                start=(zt == 0), stop=(zt == 3),
                            )
                        nc.scalar.copy(V[4 * jj + st], pv)
                    if jj < 3:
                        for t in range(4 * jj, 4 * jj + 4):
                            emit_scores(t, psS0, ppool0, width=1536)
                        finish_group(jj)  # V-scales deferred to phase 2

        # ------- phase 2+3: scores/softmax with P^T V interleaved -------
        # psS bufs=1 (4 banks) + the first 4 output s-tiles (4 banks) share
        # PSUM; out s-tiles u=0..3 accumulate during the softmax phase.
        with tc.tile_pool(name=f"p{b}", bufs=1) as ppool:
            with (
                tc.tile_pool(name=f"psS{b}", bufs=1, space="PSUM") as psS,
                tc.tile_pool(name=f"o{b}", bufs=2) as opool,
                tc.tile_pool(name=f"psPV{b}", bufs=1, space="PSUM") as psPV,
            ):
                po_tiles = {}

                def pv_open(u):
                    po_tiles[u] = psPV.tile(
                        [128, 512], F32, name=f"b{b}po{u}",
                        tag=f"o{u % 4}", space="PSUM")

                def pv_mm(u, tp):
                    # out[u-tile] += P[tp][:, u-slice].T @ V[tp]
                    nc.tensor.matmul(
                        po_tiles[u],
                        P[tp][:, 128 * u : 128 * (u + 1)],
                        V[tp],
                        start=(tp == u), stop=(tp == NT - 1),
                    )

                def pv_drain(u):
                    of = opool.tile([128, 512], F32, name=f"b{b}of{u}",
                                    tag=f"of{u % 4}")
                    nc.scalar.copy(of, po_tiles.pop(u))
                    nc.sync.dma_start(
                        out=out_d[b, 128 * u : 128 * (u + 1), :],
                        in_=of,
                    )

                # groups 0..2 V-scales ride the now-idle DVE queue
                for g in range(3):
                    scale_group(g)
                # block 0 (u=0..3): tp=0..11 spread through the 4-row window
                for u in range(4):
                    pv_open(u)
                sched = [(u, tp) for tp in range(12)
                         for u in range(min(tp + 1, 4))]
                idx = 0
                for t in range(12, NT):
                    emit_scores(t, psS, ppool, width=2048)
                    if t == NT - 1:
                        finish_group(3)
                        scale_group(3)
                    remaining = len(sched) - idx
                    take = -(-remaining // (NT - t))  # ceil, drains by row 15
                    for _ in range(take):
                        u, tp = sched[idx]
                        pv_mm(u, tp)
                        idx += 1
                # last group after scores(15)
                for tp in range(12, NT):
                    for u in range(4):
                        pv_mm(u, tp)
                for u in range(4):
                    pv_drain(u)

                # blocks 1..3 (u=4..15): all P/V resident now
                for ub in range(1, 4):
                    for u in range(4 * ub, 4 * ub + 4):
                        pv_open(u)
                        for tp in range(u, NT):
                            pv_mm(u, tp)
                        pv_drain(u)


def _host_inputs(q, W_q, W_k, W_v, W_o):
    """Build the 8 per-core input maps."""
    import ml_dtypes

    scale = 1.0 / math.sqrt(D)
    perm = np.concatenate([np.arange(0, D, 2), np.arange(1, D, 2)])

    qT = np.ascontiguousarray(
        q.reshape(B * S, D).T.astype(ml_dtypes.bfloat16))  # [D, B*S] bf16
    qT8 = np.ascontiguousarray(
        (q.reshape(B * S, D).T * 4.0).astype(ml_dtypes.float8_e4m3))

    # trig tables mirroring the reference's float32 math, rounded to bf16
    inv_freq = (1.0 / (10000.0 ** (np.arange(0, D, 2, dtype=np.float32) /
                                   np.float32(D)))).astype(np.float32)
    ang = (np.arange(S, dtype=np.float32)[:, None] * inv_freq[None, :])
    cos2 = np.ascontiguousarray(
        np.cos(ang, dtype=np.float32).T).astype(ml_dtypes.bfloat16)
    sin2 = np.ascontiguousarray(
        np.sin(ang, dtype=np.float32).T).astype(ml_dtypes.bfloat16)

    # additive triangular mask for the diagonal 128x128 block
    r = np.arange(128)[:, None]
    c = np.arange(128)[None, :]
    mask1 = np.where(c <= r, 0.0, NEG).astype(ml_dtypes.bfloat16)
    ident = np.eye(128, dtype=ml_dtypes.bfloat16)

    in_maps = []
    for h in range(NCORES):
        # fold W_o into W_v: (P^T (q W_v)) W_o == P^T (q (W_v W_o))
        wv_folded = (
            W_v[h].astype(np.float64) @ W_o[D * h : D * (h + 1), :].astype(np.float64)
        ).astype(np.float32)
        in_maps.append({
            "qT": qT,
            "qT8": qT8,
            "wq": np.ascontiguousarray(
                (W_q[h] * (scale * 2.0 ** QSHIFT / 4.0))[:, perm]
            ).astype(ml_dtypes.float8_e4m3),
            "wk": np.ascontiguousarray(
                (W_k[h] * (2.0 ** KSHIFT / 4.0))[:, perm]
            ).astype(ml_dtypes.float8_e4m3),
            "wv": wv_folded.astype(ml_dtypes.bfloat16),
            "cos2": cos2,
            "sin2": sin2,
            "mask1": mask1,
            "ident": ident,
        })
    return in_maps


def kernel(q, W_q, W_k, W_v, W_o):
    from concourse.bass_utils import run_bass_kernel_spmd

    global _BUILT
    q = np.asarray(q, dtype=np.float32)
    W_q = np.asarray(W_q, dtype=np.float32)
    W_k = np.asarray(W_k, dtype=np.float32)
    W_v = np.asarray(W_v, dtype=np.float32)
    W_o = np.asarray(W_o, dtype=np.float32)

    if _BUILT is None:
        _BUILT = build_kernel()
    nc = _BUILT

    in_maps = _host_inputs(q, W_q, W_k, W_v, W_o)
    res = run_bass_kernel_spmd(nc, in_maps, list(range(NCORES)))

    acc = np.zeros((B, S, D), dtype=np.float64)
    for h in range(NCORES):
        acc += res.results[h]["out"]
    return acc.astype(np.float32)


# revision 41
# speedup vs baseline: 1.2482x; 1.1721x over previous
"""Trainium2 Bass kernel for nn_Attention_73031623901249.

Multi-head attention with per-head 512x512 projections, interleaved RoPE,
causal softmax, a transposed P^T @ V contraction, and an output projection.

Sharding: one head per NeuronCore (H == 8 == n_cores). Each core computes its
head's full attention; the host sums the 8 partial outputs.

Structure (v6):
  - W_o folded into W_v on the host (associativity:
    (P^T (q W_v)) W_o == P^T (q (W_v W_o))) — no on-chip W_o stage.
  - q^T, W_q, W_k, W_v(folded), cos/sin are bf16; q^T and trig are loaded
    ONCE and stay resident in SBUF (no per-batch input DMAs).
  - Scores are fp8e4m3 DoubleRow matmuls (two d-tile pairs per
    instruction): rope'd Q^T/K^T are written as fp8 into [128, 2, S] pair
    tiles. Q is pre-scaled by 2^10/sqrt(D) and K by 2^6 (keeps fp8 values
    in the normal range, max ~174 < 240); the 2^-16 descale rides the Exp
    activation's scale input.
  - RoPE path: pe/po PSUM tiles are evacuated to bf16 by ACT copies, the
    four per-half products run on DVE in 2x bf16 mode, the two combines on
    GpSimd write fp8 directly.
  - P (softmax numerator) and V are bf16; the softmax denominator is folded
    into V as a per-row scale (the contraction index of P^T @ V is the
    softmax-row index). Reciprocals batched 4 rows at a time on DVE.
  - P^T V runs with P slices as the STATIONARY operand (V moving): one
    full-width [128,512] matmul per (s-tile u, z-tile t'), u <= t'. This
    4.7x's stationary reuse vs the V-stationary form, makes every matmul
    full width, and yields the output in natural [s, d] layout. The first
    4 s-tiles accumulate DURING the softmax phase (psS holds 4 PSUM banks,
    the 4 u-tiles the other 4); s-tiles 4..15 run after.

Scores rows are computed at their exact causal width; the triangular mask
for the diagonal 128-block is added on the PE (ident^T @ mask). Scores are
small by construction (|s| < ~2 after descale), so exp runs without
max-subtraction.
"""

import sys

if "/opt/trn_rl_repo" not in sys.path:
    sys.path.insert(0, "/opt/trn_rl_repo")

import math

import numpy as np

import concourse.bacc as bacc
import concourse.tile as tile
from concourse import mybir

F32 = mybir.dt.float32
BF16 = mybir.dt.bfloat16
F8 = mybir.dt.float8e4
AX = mybir.AxisListType
AF = mybir.ActivationFunctionType
DR = mybir.MatmulPerfMode.DoubleRow

B, S, D, H = 2, 2048, 512, 8
NCORES = 8
NEG = -1.0e30  # additive causal mask value

QSHIFT = 10  # Q side pre-scale 2^10 (on top of 1/sqrt(D))
KSHIFT = 6   # K side pre-scale 2^6
EXP_SCALE = 2.0 ** (-(QSHIFT + KSHIFT))

_BUILT = None


def build_kernel(reps=1):
    nc = bacc.Bacc(trn_type="TRN2", target_bir_lowering=False, debug=False)

    # inputs are host-packed so each SBUF tile fills with ONE dma (HWDGE
    # holds ~625ns per dma — count matters more than bytes)
    qT_d = nc.dram_tensor("qT", [128, 4, B * S], BF16,
                          kind="ExternalInput").ap()
    qT8_d = nc.dram_tensor("qT8", [2, 128, 2, B * S], F8,
                           kind="ExternalInput").ap()
    wq_d = nc.dram_tensor("wq", [2, 128, 2, D], F8, kind="ExternalInput").ap()
    wk_d = nc.dram_tensor("wk", [2, 128, 2, D], F8, kind="ExternalInput").ap()
    wv_d = nc.dram_tensor("wv", [128, 4, D], BF16, kind="ExternalInput").ap()
    trig_d = nc.dram_tensor("trig", [128, 4, S], BF16,
                            kind="ExternalInput").ap()
    maskid_d = nc.dram_tensor("maskid", [128, 2, 128], BF16,
                              kind="ExternalInput").ap()
    # out[b, blk, p, u, d] == out[b, s=512*blk+128*u+p, d]; host transposes
    out_d = nc.dram_tensor("out", [B, 4, 128, 4, D], F32,
                           kind="ExternalOutput").ap()

    NT = S // 128  # 16 q/key tiles per batch

    with tile.TileContext(nc) as tc:
        with tc.tile_pool(name="const", bufs=1) as constp:
            # resident q^T: fp8 z-pair tiles (Q/K proj) + bf16 (V proj)
            q8p = [constp.tile([128, 2, B * S], F8, name=f"q8p{g}")
                   for g in range(2)]
            qs_big = constp.tile([128, 4, B * S], BF16, name="qsb")
            # chunk-0 slices load first so the first projection starts ASAP
            for g in range(2):
                for m in range(2):
                    nc.sync.dma_start(
                        out=q8p[g][:, m, 0:512], in_=qT8_d[g][:, m, 0:512],
                    )
            wq8p = [constp.tile([128, 2, D], F8, name=f"wq8p{g}")
                    for g in range(2)]
            wk8p = [constp.tile([128, 2, D], F8, name=f"wk8p{g}")
                    for g in range(2)]
            wv_big = constp.tile([128, 4, D], BF16, name="wvb")
            trig_big = constp.tile([128, 4, S], BF16, name="trigb")
            trig = {("c", 0): trig_big[:, 0, :], ("c", 1): trig_big[:, 1, :],
                    ("s", 0): trig_big[:, 2, :], ("s", 1): trig_big[:, 3, :]}
            maskid = constp.tile([128, 2, 128], BF16, name="maskid")
            mask_sb = maskid[:, 0, :]
            ident_sb = maskid[:, 1, :]
            # weights on the ACT queue and trig on the GpSimd (SWDGE) queue
            # so the SP queue carries only q — three DMA streams run
            # concurrently
            for g in range(2):
                for m in range(2):
                    nc.scalar.dma_start(out=wq8p[g][:, m, :],
                                        in_=wq_d[g][:, m, :])
            nc.gpsimd.dma_start(out=trig_big[:, :, 0:512],
                                in_=trig_d[:, :, 0:512])
            for g in range(2):
                for m in range(2):
                    nc.sync.dma_start(
                        out=q8p[g][:, m, 512:S], in_=qT8_d[g][:, m, 512:S],
                    )
            nc.gpsimd.dma_start(out=trig_big[:, :, 512:S],
                                in_=trig_d[:, :, 512:S])
            nc.scalar.dma_start(out=qs_big[:, :, 0:512],
                                in_=qT_d[:, :, 0:512])
            for zt in range(4):
                nc.sync.dma_start(
                    out=qs_big[:, zt, 512:S], in_=qT_d[:, zt, 512:S],
                )

            def deferred_loads(stage):
                if stage == 0:
                    for g in range(2):
                        nc.scalar.dma_start(out=wk8p[g], in_=wk_d[g])
                elif stage == 1:
                    nc.scalar.dma_start(out=wv_big, in_=wv_d)
                    nc.gpsimd.dma_start(out=maskid, in_=maskid_d)
                    # batch-1 half of q
                    for g in range(2):
                        nc.sync.dma_start(
                            out=q8p[g][:, :, S : 2 * S],
                            in_=qT8_d[g][:, :, S : 2 * S],
                        )
                    nc.sync.dma_start(
                        out=qs_big[:, :, S : 2 * S], in_=qT_d[:, :, S : 2 * S],
                    )

            for _rep in range(reps):
                for b in range(B):
                    _build_batch(
                        nc, tc, b, q8p, qs_big, wq8p, wk8p, wv_big, trig,
                        mask_sb, ident_sb, out_d, NT,
                        deferred_loads if (_rep == 0 and b == 0) else None,
                    )
    nc.compile()
    return nc


def _build_batch(nc, tc, b, q8p, qs_big, wq8p, wk8p, wv_big, trig,
                 mask_sb, ident_sb, out_d, NT, deferred_loads=None):
    with (
        tc.tile_pool(name=f"qk{b}", bufs=1) as qkpool,
        tc.tile_pool(name=f"v{b}", bufs=1) as vpool,
        tc.tile_pool(name=f"misc{b}", bufs=1) as mpool,
        tc.tile_pool(name=f"p0{b}", bufs=1) as ppool0,
    ):
        # rope'd Q^T/K^T as fp8 pair tiles: QTp[g][:, i, :] is d-tile pair
        # member i of group g (g=0: cos-combined halves, g=1: sin-combined)
        QTp = [qkpool.tile([128, 2, S], F8, name=f"b{b}QTp{g}", tag=f"QTp{g}")
               for g in range(2)]
        KTp = [qkpool.tile([128, 2, S], F8, name=f"b{b}KTp{g}", tag=f"KTp{g}")
               for g in range(2)]
        V = [vpool.tile([128, D], BF16, name=f"b{b}V{t}", tag=f"V{t}")
             for t in range(NT)]
        rsumA = mpool.tile([128, NT], F32, name=f"b{b}rsumA")
        rsumB = mpool.tile([128, NT], F32, name=f"b{b}rsumB")
        rsum = mpool.tile([128, NT], F32, name=f"b{b}rsum")
        rinv = mpool.tile([128, NT], F32, name=f"b{b}rinv")
        # rows whose scores fit one PSUM tile never write rsumB — zero it
        nc.vector.memset(rsumB, 0.0)

        P = []

        def emit_scores(t, pool_ps, pool_p, width=1024):
            """Score row t in half-row PSUM tiles (<=2 banks each)."""
            Kt = 128 * (t + 1)
            nch = (Kt + 511) // 512
            nca = min(nch, width // 512)  # chunks in tile A
            psa = pool_ps.tile([128, width], F32, name=f"b{b}psA{t}",
                               tag="s", space="PSUM")
            psb = None
            if nch > nca:
                psb = pool_ps.tile([128, width], F32, name=f"b{b}psB{t}",
                                   tag="s", space="PSUM")

            for g in range(2):
                for c in range(nch):
                    lo, hi = 512 * c, min(512 * (c + 1), Kt)
                    tl, loc0 = (psa, lo) if c < nca else (psb, lo - width)
                    nc.tensor.matmul(
                        tl[:, loc0 : loc0 + hi - lo],
                        QTp[g][:, :, 128 * t : 128 * (t + 1)],
                        KTp[g][:, :, lo:hi],
                        start=(g == 0),
                        stop=(g == 1 and c < nch - 1),
                        perf_mode=DR,
                    )
            # additive triangular mask on the diagonal block via the PE
            mtl, mloc = (psa, Kt - 128) if nch <= nca else (psb, Kt - 128 - width)
            nc.tensor.matmul(
                mtl[:, mloc : mloc + 128], ident_sb, mask_sb,
                start=False, stop=True,
            )
            p_t = pool_p.tile([128, Kt], BF16, name=f"b{b}p{t}", tag=f"p{t}")
            nA = min(Kt, width)
            nc.scalar.activation(
                p_t[:, :nA], psa[:, :nA], AF.Exp, scale=EXP_SCALE,
                accum_out=rsumA[:, t : t + 1],
            )
            if psb is not None:
                nc.scalar.activation(
                    p_t[:, width:Kt], psb[:, : Kt - width], AF.Exp,
                    scale=EXP_SCALE, accum_out=rsumB[:, t : t + 1],
                )
            P.append(p_t)

        def finish_group(j):
            # rows 4j..4j+3 have their partial sums; combine + invert (DVE)
            g4 = slice(4 * j, 4 * j + 4)
            nc.vector.tensor_add(rsum[:, g4], rsumA[:, g4], rsumB[:, g4])
            nc.vector.reciprocal(rinv[:, g4], rsum[:, g4])

        def scale_group(j):
            # fold 1/rsum into V rows on DVE — emitted only at points where
            # the DVE queue has no pending rope work (the phase boundary /
            # softmax window), so the strict-FIFO queue never stalls rope
            for k in range(4):
                t = 4 * j + k
                nc.vector.tensor_scalar_mul(V[t], V[t], rinv[:, t : t + 1])

        # ---------------- phase 1: projections + rope ----------------
        # score rows 0..7 are emitted INSIDE phase 1 (rows t need only the
        # first t//4+1 K-chunks), soaking up ACT slack so the later
        # softmax-bound window only covers rows 8..15.
        with (
            tc.tile_pool(name=f"t{b}", bufs=2) as tpool,
            tc.tile_pool(name=f"psA{b}", bufs=2, space="PSUM") as psA,
            tc.tile_pool(name=f"psV{b}", bufs=1, space="PSUM") as psV,
            tc.tile_pool(name=f"psS0{b}", bufs=1, space="PSUM") as psS0,
        ):
            def rope(nm, dst, i, j, pe, po):
                sl = slice(512 * j, 512 * (j + 1))
                peb = tpool.tile([128, 512], BF16,
                                 name=f"peb_{b}{nm}{i}{j}", tag="peb")
                pob = tpool.tile([128, 512], BF16,
                                 name=f"pob_{b}{nm}{i}{j}", tag="pob")
                # spread PSUM evacuation: K-side po copies ride ACT (it has
                # phase-1 slack), the rest stay on DVE
                nc.vector.tensor_copy(peb, pe)
                if nm == "k":
                    nc.scalar.copy(pob, po)
                else:
                    nc.vector.tensor_copy(pob, po)
                t1 = tpool.tile([128, 512], BF16,
                                name=f"t1_{b}{nm}{i}{j}", tag="t1")
                t2 = tpool.tile([128, 512], BF16,
                                name=f"t2_{b}{nm}{i}{j}", tag="t2")
                nc.vector.tensor_mul(t1, peb, trig["c", i][:, sl])
                nc.vector.tensor_mul(t2, pob, trig["s", i][:, sl])
                t3 = tpool.tile([128, 512], BF16,
                                name=f"t3_{b}{nm}{i}{j}", tag="t3")
                t4 = tpool.tile([128, 512], BF16,
                                name=f"t4_{b}{nm}{i}{j}", tag="t4")
                nc.vector.tensor_mul(t3, peb, trig["s", i][:, sl])
                nc.vector.tensor_mul(t4, pob, trig["c", i][:, sl])
                nc.gpsimd.tensor_sub(dst[0][:, i, sl], t1, t2)
                nc.gpsimd.tensor_add(dst[1][:, i, sl], t3, t4)

            for jp in range(2):  # chunk pairs: (0,1) then (2,3)
                js = (2 * jp, 2 * jp + 1)
                qs = {jj: [qs_big[:, zt,
                                  b * S + 512 * jj : b * S + 512 * jj + 512]
                           for zt in range(4)] for jj in js}

                # Q and K projections: fp8 DoubleRow over z-pairs (their
                # only consumer is the fp8 scores path). Both chunks of the
                # pair accumulate under one stationary load per (g, d-col).
                for nm, w8p, dst in (("q", wq8p, QTp), ("k", wk8p, KTp)):
                    if deferred_loads is not None and nm == "k" and jp == 0:
                        deferred_loads(0)
                    for i in range(2):  # pair-half index
                        pe = {jj: psA.tile([128, 512], F32,
                                           name=f"b{b}{nm}pe{i}_{jj}",
                                           tag="pe", space="PSUM")
                              for jj in js}
                        po = {jj: psA.tile([128, 512], F32,
                                           name=f"b{b}{nm}po{i}_{jj}",
                                           tag="po", space="PSUM")
                              for jj in js}
                        for g in range(2):
                            for jj in js:
                                c0 = b * S + 512 * jj
                                nc.tensor.matmul(
                                    pe[jj],
                                    w8p[g][:, :, 128 * i : 128 * (i + 1)],
                                    q8p[g][:, :, c0 : c0 + 512],
                                    start=(g == 0), stop=(g == 1),
                                    perf_mode=DR,
                                )
                        for g in range(2):
                            for jj in js:
                                c0 = b * S + 512 * jj
                                nc.tensor.matmul(
                                    po[jj],
                                    w8p[g][:, :, 128 * (i + 2) : 128 * (i + 3)],
                                    q8p[g][:, :, c0 : c0 + 512],
                                    start=(g == 0), stop=(g == 1),
                                    perf_mode=DR,
                                )
                        for jj in js:
                            rope(nm, dst, i, jj, pe[jj], po[jj])

                if deferred_loads is not None and jp == 0:
                    deferred_loads(1)
                    deferred_loads = None
                # V projection (natural [s, d] layout; qT slices as
                # stationary), interleaved with the early score rows in
                # pair 0: both K-chunks are rope'd by now, so rows 0..7
                # run here and their exps soak ACT slack during pair 1
                for jj in js:
                    for st in range(4):
                        pv = psV.tile([128, 512], F32, name=f"b{b}pv{jj}_{st}",
                                      tag="pv", space="PSUM")
                        for zt in range(4):
                            nc.tensor.matmul(
                                pv, qs[jj][zt][:, 128 * st : 128 * (st + 1)],
                                wv_big[:, zt, :],
                                start=(zt == 0), stop=(zt == 3),
                            )
                        if jj == 3:
                            nc.vector.tensor_copy(V[4 * jj + st], pv)
                        else:
                            nc.scalar.copy(V[4 * jj + st], pv)
                    if jj < 3:
                        for t in range(4 * jj, 4 * jj + 4):
                            emit_scores(t, psS0, ppool0, width=1536)
                        finish_group(jj)  # V-scales deferred to phase 2

        # ------- phase 2+3: scores/softmax with P^T V interleaved -------
        # psS bufs=1 (4 banks) + the first 4 output s-tiles (4 banks) share
        # PSUM; out s-tiles u=0..3 accumulate during the softmax phase.
        with tc.tile_pool(name=f"p{b}", bufs=1) as ppool:
            with (
                tc.tile_pool(name=f"psS{b}", bufs=1, space="PSUM") as psS,
                tc.tile_pool(name=f"o{b}", bufs=2) as opool,
                tc.tile_pool(name=f"psPV{b}", bufs=1, space="PSUM") as psPV,
            ):
                po_tiles = {}

                def pv_open(u):
                    po_tiles[u] = psPV.tile(
                        [128, 512], F32, name=f"b{b}po{u}",
                        tag=f"o{u % 4}", space="PSUM")

                def pv_mm(u, tp):
                    # out[u-tile] += P[tp][:, u-slice].T @ V[tp]
                    nc.tensor.matmul(
                        po_tiles[u],
                        P[tp][:, 128 * u : 128 * (u + 1)],
                        V[tp],
                        start=(tp == u), stop=(tp == NT - 1),
                    )

                def pv_drain(u):
                    blk = u // 4
                    of = opool.tile([128, 512], F32, name=f"b{b}of{u}",
                                    tag=f"of{u % 4}")
                    nc.scalar.copy(of, po_tiles.pop(u))
                    nc.sync.dma_start(
                        out=out_d[b, blk, :, u % 4, :], in_=of,
                    )

                # groups 0..2 V-scales ride the now-idle DVE queue
                for g in range(3):
                    scale_group(g)
                # block 0 (u=0..3): tp=0..11 spread through the 4-row window
                for u in range(4):
                    pv_open(u)
                sched = [(u, tp) for tp in range(12)
                         for u in range(min(tp + 1, 4))]
                idx = 0
                for t in range(12, NT):
                    emit_scores(t, psS, ppool, width=2048)
                    if t == NT - 1:
                        finish_group(3)
                        scale_group(3)
                    remaining = len(sched) - idx
                    take = -(-remaining // (NT - t))  # ceil, drains by row 15
                    for _ in range(take):
                        u, tp = sched[idx]
                        pv_mm(u, tp)
                        idx += 1
                # last group after scores(15)
                for tp in range(12, NT):
                    for u in range(4):
                        pv_mm(u, tp)
                for u in range(4):
                    pv_drain(u)

                # blocks 1..3 (u=4..15): all P/V resident now
                for ub in range(1, 4):
                    for u in range(4 * ub, 4 * ub + 4):
                        pv_open(u)
                        for tp in range(u, NT):
                            pv_mm(u, tp)
                        pv_drain(u)


def _host_inputs(q, W_q, W_k, W_v, W_o):
    """Build the 8 per-core input maps."""
    import ml_dtypes

    scale = 1.0 / math.sqrt(D)
    perm = np.concatenate([np.arange(0, D, 2), np.arange(1, D, 2)])

    qTf = q.reshape(B * S, D).T  # [D, B*S]
    # bf16 copy packed [128, 4(zt), B*S]
    qT = np.ascontiguousarray(
        qTf.reshape(4, 128, B * S).transpose(1, 0, 2)
    ).astype(ml_dtypes.bfloat16)
    # fp8 copy (x4) packed [2(g), 128, 2(m), B*S]
    qT8 = np.ascontiguousarray(
        (qTf * 4.0).reshape(2, 2, 128, B * S).transpose(0, 2, 1, 3)
    ).astype(ml_dtypes.float8_e4m3)

    # trig tables mirroring the reference's float32 math, rounded to bf16
    inv_freq = (1.0 / (10000.0 ** (np.arange(0, D, 2, dtype=np.float32) /
                                   np.float32(D)))).astype(np.float32)
    ang = (np.arange(S, dtype=np.float32)[:, None] * inv_freq[None, :])
    cos2 = np.cos(ang, dtype=np.float32).T  # [256, S]
    sin2 = np.sin(ang, dtype=np.float32).T
    # packed [128, 4(c0,c1,s0,s1), S]
    trig = np.ascontiguousarray(np.stack(
        [cos2[:128], cos2[128:], sin2[:128], sin2[128:]], axis=1
    )).astype(ml_dtypes.bfloat16)

    # additive triangular mask for the diagonal 128x128 block
    r = np.arange(128)[:, None]
    c = np.arange(128)[None, :]
    mask1 = np.where(c <= r, 0.0, NEG).astype(np.float32)
    maskid = np.ascontiguousarray(np.stack(
        [mask1, np.eye(128, dtype=np.float32)], axis=1
    )).astype(ml_dtypes.bfloat16)

    in_maps = []
    for h in range(NCORES):
        # fold W_o into W_v: (P^T (q W_v)) W_o == P^T (q (W_v W_o))
        wv_folded = (
            W_v[h].astype(np.float64) @ W_o[D * h : D * (h + 1), :].astype(np.float64)
        ).astype(np.float32)
        wqs = (W_q[h] * (scale * 2.0 ** QSHIFT / 4.0))[:, perm]
        wks = (W_k[h] * (2.0 ** KSHIFT / 4.0))[:, perm]
        in_maps.append({
            "qT": qT,
            "qT8": qT8,
            "wq": np.ascontiguousarray(
                wqs.reshape(2, 2, 128, D).transpose(0, 2, 1, 3)
            ).astype(ml_dtypes.float8_e4m3),
            "wk": np.ascontiguousarray(
                wks.reshape(2, 2, 128, D).transpose(0, 2, 1, 3)
            ).astype(ml_dtypes.float8_e4m3),
            "wv": np.ascontiguousarray(
                wv_folded.reshape(4, 128, D).transpose(1, 0, 2)
            ).astype(ml_dtypes.bfloat16),
            "trig": trig,
            "maskid": maskid,
        })
    return in_maps


def kernel(q, W_q, W_k, W_v, W_o):
    from concourse.bass_utils import run_bass_kernel_spmd

    global _BUILT
    q = np.asarray(q, dtype=np.float32)
    W_q = np.asarray(W_q, dtype=np.float32)
    W_k = np.asarray(W_k, dtype=np.float32)
    W_v = np.asarray(W_v, dtype=np.float32)
    W_o = np.asarray(W_o, dtype=np.float32)

    if _BUILT is None:
        _BUILT = build_kernel()
    nc = _BUILT

    in_maps = _host_inputs(q, W_q, W_k, W_v, W_o)
    res = run_bass_kernel_spmd(nc, in_maps, list(range(NCORES)))

    acc = np.zeros((B, S, D), dtype=np.float64)
    for h in range(NCORES):
        o = res.results[h]["out"]  # [B, 4(blk), 128(p), 4(u), D]
        acc += o.transpose(0, 1, 3, 2, 4).reshape(B, S, D)
    return acc.astype(np.float32)
